# revision 1
# baseline (speedup 1.0000x reference)
"""Trainium2 Bass kernel for nn_CovarianceLayer.

Math (per image, PATCH=5):
    xc = center(x) - boxmean5x5(x)        # [1020, 1020]
    yc = center(y) - boxmean5x5(y)
    out = boxmean5x5(xc * yc)             # [1016, 1016]

Strategy:
  - Pure data parallel: 16 images -> 2 per NeuronCore across 8 cores.
  - Per image, process in 9 row-blocks of 128 input rows (120 output rows).
  - Horizontal pair-sums p[c] = a[c] + a[c+1] on DVE (1 pass per conv input).
  - Each 5x5 conv becomes 3 shifted banded matmuls on TensorE accumulating in
    PSUM: taps {0,1} and {3,4} come from the pair-sum at rhs offsets 0/3, tap
    {2} from the raw input at offset 2.  The vertical 5-tap band lives in the
    stationary [K, M] matrix; for the first conv the center-pixel delta is
    folded into the offset-2 band matrix, so PSUM holds xc/yc directly.
  - float32r matmuls (full fp32 storage, reduced-precision PE mode, 1 cyc/row).
  - Elementwise product xc*yc on GPSIMD, PSUM evacuations on ScalarE.
"""

import numpy as np

import concourse.bass as bass
import concourse.mybir as mybir
from concourse.tile import TileContext
from concourse.bass_utils import run_bass_kernel_spmd

PATCH = 5
H = W = 1024
ZW = W - 4          # 1020: width after first conv
OW = W - 8          # 1016: final output width
N_CORES = 8
B_TOTAL = 16
B_PER = B_TOTAL // N_CORES   # 2 images per core

R_OUT = 120         # output rows per block
XR = 128            # input rows loaded per block
ZR = R_OUT + 4      # 124 intermediate (xc/yc/z) rows per block
# block start rows (in output space); last block is shifted up so every block
# loads a full 128 input rows -- its first 64 output rows are recomputed and
# only rows [64:120) are stored.
BLOCK_STARTS = [0, 120, 240, 360, 480, 600, 720, 840, 896]

PS_C1_BUFS = 4
PS_OUT_BUFS = 4

F32 = mybir.dt.float32
F32R = mybir.dt.float32r


def _band(nrows, ncols, val):
    """W[k, m] = val for m <= k <= m+4 (vertical 5-tap band), else 0."""
    w = np.zeros((nrows, ncols), np.float32)
    for m in range(ncols):
        w[m:m + PATCH, m] = val
    return w


def _build_weights(inv_area):
    wm = _band(128, 128, -inv_area)            # -boxsum band
    wc = wm.copy()
    for m in range(128 - 2):
        wc[m + 2, m] += 1.0                    # + center-pixel delta
    wp = _band(128, 128, inv_area)             # +boxsum band (final conv)
    return wm, wc, wp


def _split_matmul_waits(nc):
    """Several walrus instruction structs (fused LDWEIGHTS+MATMUL for 4-byte
    dtypes, PSEUDO_DMA_DIRECT2D, ...) carry only one semaphore wait, while
    Tile freely attaches several.  Peel all but one wait off every
    instruction onto same-engine NoOps inserted just before it (same engine
    queue, so ordering semantics are identical)."""
    n = 0
    for f in nc.m.functions:
        for bb in f.blocks:
            i = 0
            while i < len(bb.instructions):
                inst = bb.instructions[i]
                si = inst.sync_info
                if (si is not None and len(si.on_wait) > 1
                        and not isinstance(inst, mybir.InstNoOp)):
                    extra = list(si.on_wait[:-1])
                    si.on_wait = [si.on_wait[-1]]
                    for w in extra:
                        nop = mybir.InstNoOp(name=f"I-mmwait-{n}", ins=[],
                                             outs=[])
                        n += 1
                        nop.engine = inst.engine
                        nop.sync_info = mybir.SyncInfo(on_wait=[w],
                                                       on_update=[])
                        nc.register_instruction(nop)
                        bb.instructions.insert(i, nop)
                        i += 1
                i += 1


def _build_nc():
    nc = bass.Bass()
    x_d = nc.dram_tensor("x", [B_PER, H, W], F32R, kind="ExternalInput")
    y_d = nc.dram_tensor("y", [B_PER, H, W], F32R, kind="ExternalInput")
    wm_d = nc.dram_tensor("wm", [128, 128], F32R, kind="ExternalInput")
    wc_d = nc.dram_tensor("wc", [128, 128], F32R, kind="ExternalInput")
    wp_d = nc.dram_tensor("wp", [128, 128], F32R, kind="ExternalInput")
    out_d = nc.dram_tensor("out", [B_PER, OW, OW], F32, kind="ExternalOutput")

    with TileContext(nc) as tc:
        with (
            tc.tile_pool(name="consts", bufs=1) as cpool,
            tc.tile_pool(name="io", bufs=5) as iopool,
            tc.tile_pool(name="work", bufs=4) as wpool,
            tc.tile_pool(name="ps_c1", bufs=PS_C1_BUFS, space="PSUM") as ps_c1,
            tc.tile_pool(name="ps_out", bufs=PS_OUT_BUFS, space="PSUM") as ps_out,
        ):
            wm_t = cpool.tile([128, 128], F32R)
            wc_t = cpool.tile([128, 128], F32R)
            wp_t = cpool.tile([128, 128], F32R)
            nc.sync.dma_start(out=wm_t[:, :], in_=wm_d[:, :])
            nc.sync.dma_start(out=wc_t[:, :], in_=wc_d[:, :])
            nc.sync.dma_start(out=wp_t[:, :], in_=wp_d[:, :])

            for b in range(B_PER):
                for s in BLOCK_STARTS:
                    xt = iopool.tile([XR, W], F32R, tag="xt")
                    yt = iopool.tile([XR, W], F32R, tag="yt")
                    nc.gpsimd.dma_start(out=xt[:, :], in_=x_d[b, s:s + XR, :])
                    nc.sync.dma_start(out=yt[:, :], in_=y_d[b, s:s + XR, :])

                    # horizontal pair sums  p[c] = a[c] + a[c+1]
                    px = wpool.tile([XR, W - 1], F32R, tag="px")
                    py = wpool.tile([XR, W - 1], F32R, tag="py")
                    nc.vector.tensor_add(out=px[:, :], in0=xt[:, 0:W - 1].bitcast(F32),
                                         in1=xt[:, 1:W].bitcast(F32))
                    nc.vector.tensor_add(out=py[:, :], in0=yt[:, 0:W - 1].bitcast(F32),
                                         in1=yt[:, 1:W].bitcast(F32))

                    # first conv: xc/yc = center - boxmean, via 3 banded
                    # matmuls per 512-column PSUM bank
                    xc_sb = wpool.tile([ZR, ZW], F32, tag="xc")
                    yc_sb = wpool.tile([ZR, ZW], F32, tag="yc")
                    for pt, raw, dst in ((px, xt, xc_sb), (py, yt, yc_sb)):
                        for c0, c1 in ((0, 512), (512, ZW)):
                            cps = ps_c1.tile([ZR, 512], F32, tag="c1")
                            n = c1 - c0
                            nc.tensor.matmul(
                                cps[:, :n],
                                wm_t[:XR, :ZR],
                                pt[:, c0:c1],
                                start=True, stop=False)
                            nc.tensor.matmul(
                                cps[:, :n],
                                wm_t[:XR, :ZR],
                                pt[:, 3 + c0:3 + c1],
                                start=False, stop=False)
                            nc.tensor.matmul(
                                cps[:, :n],
                                wc_t[:XR, :ZR],
                                raw[:, 2 + c0:2 + c1],
                                start=False, stop=True)
                            nc.scalar.copy(out=dst[:, c0:c1], in_=cps[:, :n])

                    # elementwise covariance term
                    z = wpool.tile([ZR, ZW], F32R, tag="z")
                    nc.gpsimd.tensor_mul(out=z[:, :], in0=xc_sb[:, :],
                                         in1=yc_sb[:, :])
                    pz = wpool.tile([ZR, ZW - 1], F32R, tag="pz")
                    nc.vector.tensor_add(out=pz[:, :], in0=z[:, 0:ZW - 1].bitcast(F32),
                                         in1=z[:, 1:ZW].bitcast(F32))

                    # final conv: out = boxmean(z)
                    out_sb = wpool.tile([R_OUT, OW], F32, tag="out_sb")
                    for c0, c1 in ((0, 512), (512, OW)):
                        ops = ps_out.tile([R_OUT, 512], F32, tag="po")
                        n = c1 - c0
                        nc.tensor.matmul(
                            ops[:, :n],
                            wp_t[:ZR, :R_OUT],
                            pz[:, c0:c1],
                            start=True, stop=False)
                        nc.tensor.matmul(
                            ops[:, :n],
                            wp_t[:ZR, :R_OUT],
                            pz[:, 2 + c0:2 + c1],
                            start=False, stop=False)
                        nc.tensor.matmul(
                            ops[:, :n],
                            wp_t[:ZR, :R_OUT],
                            z[:, 4 + c0:4 + c1],
                            start=False, stop=True)
                        nc.scalar.copy(out=out_sb[:, c0:c1], in_=ops[:, :n])

                    if s == BLOCK_STARTS[-1]:
                        # overlapped tail block: only store the fresh rows
                        skip = BLOCK_STARTS[-2] + R_OUT - s   # 64
                        nc.sync.dma_start(out=out_d[b, s + skip:s + R_OUT, :],
                                          in_=out_sb[skip:, :])
                    else:
                        nc.sync.dma_start(out=out_d[b, s:s + R_OUT, :],
                                          in_=out_sb[:, :])
    _split_matmul_waits(nc)
    return nc


def kernel(x, y, mean_mask, ones_mask):
    x = np.ascontiguousarray(np.asarray(x, np.float32).reshape(B_TOTAL, H, W))
    y = np.ascontiguousarray(np.asarray(y, np.float32).reshape(B_TOTAL, H, W))
    inv_area = float(np.asarray(mean_mask).reshape(-1)[0])   # 1/25
    wm, wc, wp = _build_weights(inv_area)

    nc = _build_nc()
    in_maps = []
    for c in range(N_CORES):
        in_maps.append({
            "x": np.ascontiguousarray(x[c * B_PER:(c + 1) * B_PER]),
            "y": np.ascontiguousarray(y[c * B_PER:(c + 1) * B_PER]),
            "wm": wm, "wc": wc, "wp": wp,
        })
    res = run_bass_kernel_spmd(nc, in_maps, list(range(N_CORES)))
    out = np.concatenate([r["out"] for r in res.results], axis=0)
    return out.reshape(B_TOTAL, 1, OW, OW).astype(np.float32)



# revision 3
# speedup vs baseline: 1.0333x; 1.0333x over previous
"""Trainium2 Bass kernel for nn_CovarianceLayer (v2).

Math (per image, PATCH=5):
    xc = center(x) - boxmean5x5(x)        # [1020, 1020]
    yc = center(y) - boxmean5x5(y)
    out = boxmean5x5(xc * yc)             # [1016, 1016]

Strategy (per core: 2 images, 9 row-blocks of 128 input rows):
  - fp16 I/O: host converts x,y to fp16 and upconverts the fp16 output.
  - conv1 per field = 3 fp16 tensor-engine streams per 512-col chunk:
    pair-sums p = a[c]+a[c+1] at column offsets 0 and 3 under a banded
    5-tap vertical stationary, plus the raw input at offset 2 carrying the
    remaining boxmean taps and the +1 center-pixel delta.  PSUM fp32.
  - Only one PSUM operand is allowed per vector op, so Act evacuates xc
    chunks to fp16 SBUF and DVE multiplies them against ps_y -> z fp16.
  - conv2 = single stream on the fully pre-summed hz (pz/qz/hz fp16 adds
    spread across DVE and GpSimd), band stationary carries the 1/25.
  - Engine split: SP queue loads; Act evacuates + shares stores with SP;
    GpSimd does px, py, qz; DVE does mult, pz, hz.  PE ~54us critical.
"""

import numpy as np

import concourse.bass as bass
import concourse.mybir as mybir
from concourse.tile import TileContext
from concourse.bass_utils import run_bass_kernel_spmd

PATCH = 5
H = W = 1024
ZW = W - 4          # 1020
OW = W - 8          # 1016
N_CORES = 8
B_TOTAL = 16
B_PER = B_TOTAL // N_CORES

R_OUT = 120
XR = 128
ZR = R_OUT + 4      # 124
N_TILES = 17        # 2 images as one 2048-row virtual strip: 120*16+128=2048

F32 = mybir.dt.float32
F16 = mybir.dt.float16


def _build_weights():
    inv16 = np.float32(np.float16(1.0 / 25.0))
    # pair-sum stream stationary: -1/25 vertical 5-tap band
    wb = np.zeros((128, ZR), np.float32)
    for m in range(ZR):
        wb[m:m + PATCH, m] = -inv16
    # center stream: -1/25 band (h-col 2) + center delta
    wc = wb.copy()
    for m in range(ZR):
        wc[m + 2, m] += 1.0
    # conv2 band: +1/25, [ZR rows, R_OUT cols]
    wp = np.zeros((ZR, R_OUT), np.float32)
    for m in range(R_OUT):
        wp[m:m + PATCH, m] = inv16
    return wb.astype(np.float16), wc.astype(np.float16), wp.astype(np.float16)


def _split_matmul_waits(nc):
    """Peel all but one semaphore wait off every instruction onto same-engine
    NoOps (walrus instruction structs carry only one wait)."""
    n = 0
    for f in nc.m.functions:
        for bb in f.blocks:
            i = 0
            while i < len(bb.instructions):
                inst = bb.instructions[i]
                si = inst.sync_info
                if (si is not None and len(si.on_wait) > 1
                        and not isinstance(inst, mybir.InstNoOp)):
                    extra = list(si.on_wait[:-1])
                    si.on_wait = [si.on_wait[-1]]
                    for w in extra:
                        nop = mybir.InstNoOp(name=f"I-mmwait-{n}", ins=[],
                                             outs=[])
                        n += 1
                        nop.engine = inst.engine
                        nop.sync_info = mybir.SyncInfo(on_wait=[w],
                                                       on_update=[])
                        nc.register_instruction(nop)
                        bb.instructions.insert(i, nop)
                        i += 1
                i += 1


def _build_nc():
    nc = bass.Bass()
    x_d = nc.dram_tensor("x", [B_PER, H, W], F16, kind="ExternalInput")
    y_d = nc.dram_tensor("y", [B_PER, H, W], F16, kind="ExternalInput")
    wb_d = nc.dram_tensor("wb", [128, ZR], F16, kind="ExternalInput")
    wc_d = nc.dram_tensor("wc", [128, ZR], F16, kind="ExternalInput")
    wp_d = nc.dram_tensor("wp", [ZR, R_OUT], F16, kind="ExternalInput")
    out_d = nc.dram_tensor("out", [B_PER, OW, OW], F16, kind="ExternalOutput")

    with TileContext(nc) as tc:
        with (
            tc.tile_pool(name="consts", bufs=1) as cpool,
            tc.tile_pool(name="io", bufs=5) as iopool,
            tc.tile_pool(name="work", bufs=4) as wpool,
            tc.tile_pool(name="ps_c1", bufs=3, space="PSUM") as ps_c1,
            tc.tile_pool(name="ps_out", bufs=2, space="PSUM") as ps_out,
        ):
            wb_t = cpool.tile([128, ZR], F16)
            wc_t = cpool.tile([128, ZR], F16)
            wp_t = cpool.tile([ZR, R_OUT], F16)
            nc.gpsimd.dma_start(out=wb_t[:, :], in_=wb_d[:, :])
            nc.gpsimd.dma_start(out=wc_t[:, :], in_=wc_d[:, :])
            nc.gpsimd.dma_start(out=wp_t[:, :], in_=wp_d[:, :])

            for t in range(N_TILES):
                    s = 120 * t        # virtual input row base
                    xt = iopool.tile([XR, W], F16, tag="xt")
                    yt = iopool.tile([XR, W], F16, tag="yt")
                    # load input rows [s, s+128) of the 2-image virtual strip
                    for src_d, dst, q in ((x_d, xt, nc.sync), (y_d, yt,
                                          nc.scalar if t == 0 else nc.sync)):
                        if t == 0:
                            # column-split so the first conv1 chunk can start
                            # as soon as the left half + its pair-sum land
                            q.dma_start(out=dst[:, 0:640],
                                        in_=src_d[0, s:s + XR, 0:640])
                            q.dma_start(out=dst[:, 640:W],
                                        in_=src_d[0, s:s + XR, 640:W])
                        elif s + XR <= H:
                            q.dma_start(out=dst[:, :], in_=src_d[0, s:s + XR, :])
                        elif s >= H:
                            q.dma_start(out=dst[:, :],
                                        in_=src_d[1, s - H:s - H + XR, :])
                        else:
                            n0 = H - s
                            q.dma_start(out=dst[0:n0, :], in_=src_d[0, s:H, :])
                            q.dma_start(out=dst[n0:XR, :],
                                        in_=src_d[1, 0:XR - n0, :])

                    # horizontal pair sums (GpSimd, fp16)
                    px = wpool.tile([XR, W - 1], F16, tag="px")
                    py = wpool.tile([XR, W - 1], F16, tag="py")
                    if t == 0:
                        nc.gpsimd.tensor_add(out=px[:, 0:639],
                                             in0=xt[:, 0:639], in1=xt[:, 1:640])
                        nc.gpsimd.tensor_add(out=px[:, 639:W - 1],
                                             in0=xt[:, 639:W - 1], in1=xt[:, 640:W])
                    else:
                        nc.gpsimd.tensor_add(out=px[:, :], in0=xt[:, 0:W - 1],
                                             in1=xt[:, 1:W])
                    nc.gpsimd.tensor_add(out=py[:, :], in0=yt[:, 0:W - 1],
                                         in1=yt[:, 1:W])

                    # conv1 per 512-col chunk: 3 streams into PSUM; Act
                    # evacuates xc to fp16, DVE multiplies against ps_y
                    z = wpool.tile([ZR, ZW], F16, tag="z")
                    ex = wpool.tile([ZR, ZW], F16, tag="ex")
                    for c0, c1 in ((0, 512), (512, ZW)):
                        n = c1 - c0
                        ps_x = ps_c1.tile([ZR, 512], F32, tag="psx")
                        ps_y = ps_c1.tile([ZR, 512], F32, tag="psy")
                        for pt, raw, cps in ((px, xt, ps_x), (py, yt, ps_y)):
                            nc.tensor.matmul(
                                cps[:, :n], wb_t[:, :], pt[:, c0:c1],
                                start=True, stop=False)
                            nc.tensor.matmul(
                                cps[:, :n], wb_t[:, :], pt[:, 3 + c0:3 + c1],
                                start=False, stop=False)
                            nc.tensor.matmul(
                                cps[:, :n], wc_t[:, :], raw[:, 2 + c0:2 + c1],
                                start=False, stop=True)
                        nc.scalar.copy(out=ex[:, c0:c1], in_=ps_x[:, :n])
                        nc.vector.tensor_mul(out=z[:, c0:c1], in0=ex[:, c0:c1],
                                             in1=ps_y[:, :n])

                    # horizontal 5-sum of z: pz (DVE), qz (GpSimd), hz (DVE).
                    # Last tile: column-split each op across DVE+GpSimd so the
                    # epilogue chain drains faster.
                    pz = wpool.tile([ZR, ZW - 1], F16, tag="pz")
                    qz = wpool.tile([ZR, ZW - 3], F16, tag="qz")
                    hz = wpool.tile([ZR, OW], F16, tag="hz")
                    if t < N_TILES - 2:
                        nc.vector.tensor_add(out=pz[:, :], in0=z[:, 0:ZW - 1],
                                             in1=z[:, 1:ZW])
                        nc.gpsimd.tensor_add(out=qz[:, :], in0=pz[:, 0:ZW - 3],
                                             in1=pz[:, 2:ZW - 1])
                        nc.vector.tensor_add(out=hz[:, :], in0=qz[:, 0:OW],
                                             in1=z[:, 4:ZW])
                    else:
                        m = 512
                        nc.vector.tensor_add(out=pz[:, 0:m], in0=z[:, 0:m],
                                             in1=z[:, 1:m + 1])
                        nc.gpsimd.tensor_add(out=pz[:, m:], in0=z[:, m:ZW - 1],
                                             in1=z[:, m + 1:ZW])
                        nc.vector.tensor_add(out=qz[:, 0:m], in0=pz[:, 0:m],
                                             in1=pz[:, 2:m + 2])
                        nc.gpsimd.tensor_add(out=qz[:, m:], in0=pz[:, m:ZW - 3],
                                             in1=pz[:, m + 2:ZW - 1])
                        nc.vector.tensor_add(out=hz[:, 0:m], in0=qz[:, 0:m],
                                             in1=z[:, 4:m + 4])
                        nc.gpsimd.tensor_add(out=hz[:, m:], in0=qz[:, m:OW],
                                             in1=z[:, m + 4:ZW])

                    # conv2: single stream on hz; chunked double-buffered PSUM
                    out_sb = wpool.tile([R_OUT, OW], F16, tag="osb")
                    for ci, (c0, c1) in enumerate(((0, 512), (512, OW))):
                        ops = ps_out.tile([R_OUT, 512], F32, tag="po")
                        nc.tensor.matmul(ops[:, 0:c1 - c0], wp_t[:, :],
                                         hz[:, c0:c1], start=True, stop=True)
                        if t == N_TILES - 1 and ci == 1:
                            # drain the tail in parallel: DVE evacuates the
                            # second chunk while Act does the first; stores
                            # go per-chunk on otherwise-idle queues
                            nc.vector.tensor_scalar_add(
                                out=out_sb[:, c0:c1], in0=ops[:, 0:c1 - c0],
                                scalar1=0.0)
                        else:
                            nc.scalar.copy(out=out_sb[:, c0:c1],
                                           in_=ops[:, 0:c1 - c0])
                        if t == N_TILES - 1:
                            sq = nc.sync if ci == 0 else nc.gpsimd
                            sq.dma_start(
                                out=out_d[1, s - H:s - H + R_OUT, c0:c1],
                                in_=out_sb[:, c0:c1])

                    # store valid out rows (virtual rows [s, s+120); rows
                    # 1016..1023 of the virtual strip are cross-image garbage)
                    if t == N_TILES - 1:
                        continue        # stored per-chunk above
                    store_q = nc.scalar if t % 4 == 3 else nc.sync
                    if s + R_OUT <= OW:
                        store_q.dma_start(out=out_d[0, s:s + R_OUT, :],
                                          in_=out_sb[:, :])
                    elif s >= H:
                        store_q.dma_start(out=out_d[1, s - H:s - H + R_OUT, :],
                                          in_=out_sb[:, :])
                    else:
                        n0 = OW - s                     # img0 rows in this tile
                        k1 = H - s                      # tile row of img1 row 0
                        store_q.dma_start(out=out_d[0, s:OW, :],
                                          in_=out_sb[0:n0, :])
                        store_q.dma_start(out=out_d[1, 0:R_OUT - k1, :],
                                          in_=out_sb[k1:R_OUT, :])
    _split_matmul_waits(nc)
    return nc


def kernel(x, y, mean_mask, ones_mask):
    x16 = np.ascontiguousarray(
        np.asarray(x, np.float32).reshape(B_TOTAL, H, W).astype(np.float16))
    y16 = np.ascontiguousarray(
        np.asarray(y, np.float32).reshape(B_TOTAL, H, W).astype(np.float16))
    wb, wc, wp = _build_weights()

    nc = _build_nc()
    in_maps = []
    for c in range(N_CORES):
        in_maps.append({
            "x": np.ascontiguousarray(x16[c * B_PER:(c + 1) * B_PER]),
            "y": np.ascontiguousarray(y16[c * B_PER:(c + 1) * B_PER]),
            "wb": wb, "wc": wc, "wp": wp,
        })
    res = run_bass_kernel_spmd(nc, in_maps, list(range(N_CORES)))
    out = np.concatenate([r["out"] for r in res.results], axis=0)
    return out.reshape(B_TOTAL, 1, OW, OW).astype(np.float32)


# revision 5
# speedup vs baseline: 1.0628x; 1.0286x over previous
"""Trainium2 Bass kernel for nn_CovarianceLayer (v3: 2-stream conv1 with
front/mid software pipelining).

conv1 per field = 2 streams: 4-tap sum q = p + p<<3 (h-taps 0,1,3,4) under
the -1/25 band + raw@2 (box tap + center delta).  Front stages (loads,
pair-sums, q-sums) of tile t+1 are emitted before the middle stages of tile
t so DVE serves qx(t+1) before mult(t), keeping PE fed.
"""

import numpy as np

import concourse.bass as bass
import concourse.mybir as mybir
from concourse.tile import TileContext
from concourse.bass_utils import run_bass_kernel_spmd

PATCH = 5
H = W = 1024
ZW = W - 4          # 1020
OW = W - 8          # 1016
N_CORES = 8
B_TOTAL = 16
B_PER = B_TOTAL // N_CORES

R_OUT = 120
XR = 128
ZR = R_OUT + 4      # 124
N_TILES = 17        # 2 images as one 2048-row virtual strip: 120*16+128=2048

F32 = mybir.dt.float32
F16 = mybir.dt.float16


def _build_weights():
    inv16 = np.float32(np.float16(1.0 / 25.0))
    wb = np.zeros((128, ZR), np.float32)
    for m in range(ZR):
        wb[m:m + PATCH, m] = -inv16
    wc = wb.copy()
    for m in range(ZR):
        wc[m + 2, m] += 1.0
    wp = np.zeros((ZR, R_OUT), np.float32)
    for m in range(R_OUT):
        wp[m:m + PATCH, m] = inv16
    return wb.astype(np.float16), wc.astype(np.float16), wp.astype(np.float16)


def _split_matmul_waits(nc):
    n = 0
    for f in nc.m.functions:
        for bb in f.blocks:
            i = 0
            while i < len(bb.instructions):
                inst = bb.instructions[i]
                si = inst.sync_info
                if (si is not None and len(si.on_wait) > 1
                        and not isinstance(inst, mybir.InstNoOp)):
                    extra = list(si.on_wait[:-1])
                    si.on_wait = [si.on_wait[-1]]
                    for w in extra:
                        nop = mybir.InstNoOp(name=f"I-mmwait-{n}", ins=[],
                                             outs=[])
                        n += 1
                        nop.engine = inst.engine
                        nop.sync_info = mybir.SyncInfo(on_wait=[w],
                                                       on_update=[])
                        nc.register_instruction(nop)
                        bb.instructions.insert(i, nop)
                        i += 1
                i += 1


def _build_nc():
    nc = bass.Bass()
    x_d = nc.dram_tensor("x", [B_PER, H, W], F16, kind="ExternalInput")
    y_d = nc.dram_tensor("y", [B_PER, H, W], F16, kind="ExternalInput")
    wb_d = nc.dram_tensor("wb", [128, ZR], F16, kind="ExternalInput")
    wc_d = nc.dram_tensor("wc", [128, ZR], F16, kind="ExternalInput")
    wp_d = nc.dram_tensor("wp", [ZR, R_OUT], F16, kind="ExternalInput")
    out_d = nc.dram_tensor("out", [B_PER, OW, OW], F16, kind="ExternalOutput")

    with TileContext(nc) as tc:
        with (
            tc.tile_pool(name="consts", bufs=1) as cpool,
            tc.tile_pool(name="io", bufs=6) as iopool,
            tc.tile_pool(name="work", bufs=5) as wpool,
            tc.tile_pool(name="ps_c1", bufs=3, space="PSUM") as ps_c1,
            tc.tile_pool(name="ps_out", bufs=2, space="PSUM") as ps_out,
        ):
            wb_t = cpool.tile([128, ZR], F16)
            wc_t = cpool.tile([128, ZR], F16)
            wp_t = cpool.tile([ZR, R_OUT], F16)
            nc.scalar.dma_start(out=wb_t[:, :], in_=wb_d[:, :])
            nc.scalar.dma_start(out=wc_t[:, :], in_=wc_d[:, :])
            nc.scalar.dma_start(out=wp_t[:, :], in_=wp_d[:, :])

            def emit_front(t):
                s = 120 * t
                xt = iopool.tile([XR, W], F16, tag="xt")
                yt = iopool.tile([XR, W], F16, tag="yt")
                for src_d, dst, q in ((x_d, xt, nc.sync), (y_d, yt,
                                      nc.gpsimd if t == 0 else nc.sync)):
                    if t == 0:
                        q.dma_start(out=dst[:, 0:640],
                                    in_=src_d[0, s:s + XR, 0:640])
                        q.dma_start(out=dst[:, 640:W],
                                    in_=src_d[0, s:s + XR, 640:W])
                    elif s + XR <= H:
                        q.dma_start(out=dst[:, :], in_=src_d[0, s:s + XR, :])
                    elif s >= H:
                        q.dma_start(out=dst[:, :],
                                    in_=src_d[1, s - H:s - H + XR, :])
                    else:
                        n0 = H - s
                        q.dma_start(out=dst[0:n0, :], in_=src_d[0, s:H, :])
                        q.dma_start(out=dst[n0:XR, :],
                                    in_=src_d[1, 0:XR - n0, :])

                px = wpool.tile([XR, W - 1], F16, tag="px")
                py = wpool.tile([XR, W - 1], F16, tag="py")
                if t == 0:
                    nc.gpsimd.tensor_add(out=px[:, 0:639], in0=xt[:, 0:639],
                                         in1=xt[:, 1:640])
                    nc.gpsimd.tensor_add(out=px[:, 639:W - 1],
                                         in0=xt[:, 639:W - 1],
                                         in1=xt[:, 640:W])
                else:
                    nc.gpsimd.tensor_add(out=px[:, 0:460], in0=xt[:, 0:460],
                                         in1=xt[:, 1:461])
                    nc.vector.tensor_add(out=px[:, 460:W - 1],
                                         in0=xt[:, 460:W - 1],
                                         in1=xt[:, 461:W])
                nc.gpsimd.tensor_add(out=py[:, :], in0=yt[:, 0:W - 1],
                                     in1=yt[:, 1:W])
                qx = wpool.tile([XR, ZW], F16, tag="qx")
                qy = wpool.tile([XR, ZW], F16, tag="qy")
                nc.vector.tensor_add(out=qx[:, :], in0=px[:, 0:ZW],
                                     in1=px[:, 3:3 + ZW])
                nc.vector.tensor_add(out=qy[:, :], in0=py[:, 0:ZW],
                                     in1=py[:, 3:3 + ZW])
                return (t, s, xt, yt, qx, qy)

            def emit_mid(t, s, xt, yt, qx, qy):
                z = wpool.tile([ZR, ZW], F16, tag="z")
                ex = wpool.tile([ZR, ZW], F16, tag="ex")
                for c0, c1 in ((0, 512), (512, ZW)):
                    n = c1 - c0
                    ps_x = ps_c1.tile([ZR, 512], F32, tag="psx")
                    ps_y = ps_c1.tile([ZR, 512], F32, tag="psy")
                    for qt, raw, cps in ((qx, xt, ps_x), (qy, yt, ps_y)):
                        nc.tensor.matmul(cps[:, :n], wb_t[:, :], qt[:, c0:c1],
                                         start=True, stop=False)
                        nc.tensor.matmul(cps[:, :n], wc_t[:, :],
                                         raw[:, 2 + c0:2 + c1],
                                         start=False, stop=True)
                    nc.scalar.copy(out=ex[:, c0:c1], in_=ps_x[:, :n])
                    nc.vector.tensor_mul(out=z[:, c0:c1], in0=ex[:, c0:c1],
                                         in1=ps_y[:, :n])

                pz = wpool.tile([ZR, ZW - 1], F16, tag="pz")
                tz = wpool.tile([ZR, ZW - 2], F16, tag="tz")
                if t < N_TILES - 2:
                    nc.gpsimd.tensor_add(out=pz[:, :], in0=z[:, 0:ZW - 1],
                                         in1=z[:, 1:ZW])
                    nc.gpsimd.tensor_add(out=tz[:, :], in0=pz[:, 0:ZW - 2],
                                         in1=z[:, 2:ZW])
                else:
                    m = 512
                    nc.vector.tensor_add(out=pz[:, 0:m], in0=z[:, 0:m],
                                         in1=z[:, 1:m + 1])
                    nc.gpsimd.tensor_add(out=pz[:, m:], in0=z[:, m:ZW - 1],
                                         in1=z[:, m + 1:ZW])
                    nc.vector.tensor_add(out=tz[:, 0:m], in0=pz[:, 0:m],
                                         in1=z[:, 2:m + 2])
                    nc.gpsimd.tensor_add(out=tz[:, m:], in0=pz[:, m:ZW - 2],
                                         in1=z[:, m + 2:ZW])

                out_sb = wpool.tile([R_OUT, OW], F16, tag="osb")
                for ci, (c0, c1) in enumerate(((0, 512), (512, OW))):
                    ops = ps_out.tile([R_OUT, 512], F32, tag="po")
                    nc.tensor.matmul(ops[:, 0:c1 - c0], wp_t[:, :],
                                     tz[:, c0:c1], start=True, stop=False)
                    nc.tensor.matmul(ops[:, 0:c1 - c0], wp_t[:, :],
                                     pz[:, 3 + c0:3 + c1],
                                     start=False, stop=True)
                    if t == N_TILES - 1 and ci == 1:
                        nc.vector.tensor_scalar_add(
                            out=out_sb[:, c0:c1], in0=ops[:, 0:c1 - c0],
                            scalar1=0.0)
                    else:
                        nc.scalar.copy(out=out_sb[:, c0:c1],
                                       in_=ops[:, 0:c1 - c0])
                    if t == N_TILES - 1:
                        sq = nc.sync if ci == 0 else nc.gpsimd
                        sq.dma_start(out=out_d[1, s - H:s - H + R_OUT, c0:c1],
                                     in_=out_sb[:, c0:c1])
                if t == N_TILES - 1:
                    return
                if s + R_OUT <= OW:
                    nc.sync.dma_start(out=out_d[0, s:s + R_OUT, :],
                                      in_=out_sb[:, :])
                elif s >= H:
                    nc.sync.dma_start(out=out_d[1, s - H:s - H + R_OUT, :],
                                      in_=out_sb[:, :])
                else:
                    n0 = OW - s
                    k1 = H - s
                    nc.sync.dma_start(out=out_d[0, s:OW, :],
                                      in_=out_sb[0:n0, :])
                    nc.sync.dma_start(out=out_d[1, 0:R_OUT - k1, :],
                                      in_=out_sb[k1:R_OUT, :])

            # software pipeline: front(t+1) emitted before mid(t)
            pending = None
            for t in range(N_TILES):
                f = emit_front(t)
                if pending is not None:
                    emit_mid(*pending)
                pending = f
            emit_mid(*pending)
    _split_matmul_waits(nc)
    return nc


def kernel(x, y, mean_mask, ones_mask):
    x16 = np.ascontiguousarray(
        np.asarray(x, np.float32).reshape(B_TOTAL, H, W).astype(np.float16))
    y16 = np.ascontiguousarray(
        np.asarray(y, np.float32).reshape(B_TOTAL, H, W).astype(np.float16))
    wb, wc, wp = _build_weights()

    nc = _build_nc()
    in_maps = []
    for c in range(N_CORES):
        in_maps.append({
            "x": np.ascontiguousarray(x16[c * B_PER:(c + 1) * B_PER]),
            "y": np.ascontiguousarray(y16[c * B_PER:(c + 1) * B_PER]),
            "wb": wb, "wc": wc, "wp": wp,
        })
    res = run_bass_kernel_spmd(nc, in_maps, list(range(N_CORES)))
    out = np.concatenate([r["out"] for r in res.results], axis=0)
    return out.reshape(B_TOTAL, 1, OW, OW).astype(np.float32)
